# revision 27
# baseline (speedup 1.0000x reference)
# Trainium2 Bass kernel for residual-VQ autoencoder loss (vq_codebook).
# Data-parallel over rows: 8 NeuronCores, 2048 rows each; codebooks/weights
# replicated. The scalar loss is assembled on host from per-core [128,40]
# partial-sum outputs (no on-chip collectives needed).
#
# Per core (RT=16 row-tiles of 128):
#   encoder: h = x@W1+b1 -> LN -> ReLU -> latent = h@W2' + b2 (ln_g folded into W2)
#   RVQ: psum_score[r,v] = 2K * r . E_v  (bf16 matmul, pre-scaled E^T)
#     one custom-DVE pass per [128,2048] PSUM group adds K*(SHIFT-|E_v|^2),
#     quantizes via (x+BIG)-BIG to multiples of 8192, adds the global vocab
#     index, and MAX-accumulates (chained across 4 groups) => packed argmax.
#     idx = packed mod 8192; q = codebook[idx] via gpsimd.dma_gather (exact f32);
#     resid -= q^T (PE transpose + DVE sub).
#   loss telescopes: sum_l 1.5*mean((q_l-r_l)^2) = 1.5*(R0-R4)/(N*LAT);
#   quant^T = latent^T - resid^T; decoder stays feature-major (no transposes);
#   Rrec = sum((recon-x)^2) using x^T spilled to DRAM during the encoder.

import sys

sys.path.insert(0, "/opt/trn_rl_repo")

import numpy as np

import concourse.bass as bass
import concourse.mybir as mybir
import concourse.bacc as bacc
import concourse.tile as tile
from concourse.alu_op_type import AluOpType
from concourse.bass_utils import run_bass_kernel_spmd

OBS, HID, LAT = 1024, 2048, 256
VOCAB, HQ = 8192, 4
N, NCORES = 16384, 8
NSH = N // NCORES          # 2048 rows per core
RT = NSH // 128            # 16 row tiles
LN_EPS = 1e-5
GRID = 2048.0               # one vocab group per QPACK call; 11 index bits
BIG = float(1.5 * 2.0**34)  # ulp(BIG)=2048 -> (x+BIG)-BIG rounds to mult of GRID

f32 = mybir.dt.float32
f32r = mybir.dt.float32r
bf16 = mybir.dt.bfloat16
i16 = mybir.dt.int16

_QPACK = None


def _register_qpack():
    """out = (((Src0+Src1)+BIG)-BIG) + Idx; accum_out = max over free dim.
    Src0: PSUM scores 2K*r.E (f32); Src1: K*(SHIFT-|E|^2) replicated (bf16).
    Quantizes the score to multiples of GRID and packs the within-group index
    into the low bits; per-group accums are combined at level end."""
    global _QPACK
    if _QPACK is not None:
        return _QPACK
    from concourse import dve_ops
    from concourse.dve_spec import Spec, Src0, Src1, C2, AluOp, Idx, lower
    from concourse.dve_table_gen import DveOpSpec

    def _ref(in0, in1, s0, s1, imm2):
        x = np.asarray(in0, np.float32) + np.asarray(in1, np.float32)
        q = (x + np.float32(imm2)).astype(np.float32) - np.float32(imm2)
        idxv = np.arange(x.shape[-1], dtype=np.float32)
        out = (q + idxv).astype(np.float32)
        acc = out.max(axis=-1, keepdims=True)
        return out, acc

    body = (((Src0 + Src1) + C2) - C2) + Idx
    spec = Spec(body=body, accum=AluOp.MAX, reference=_ref)
    op = dve_ops.DveOp("QPACK_ARGMAX", spec, subdim=False, uops_sha={})
    dve_ops.OPS.append(op)
    dve_ops.CUSTOM_DVE_SPECS[op.name] = op.spec
    dve_ops._SUB_OPCODE_FOR_NAME[op.name] = (
        dve_ops._CUSTOM_DVE_ROW_BASE + len(dve_ops.OPS) - 1
    )
    for ver in ("v3", "v4"):
        s = DveOpSpec(
            name=op.name,
            opcode=dve_ops.get_dve_sub_opcode(op.name),
            uops=lower(spec, ver=ver),
            rd1_en=True,
        )
        op.uops_sha[ver] = s.sha(ver)
    _QPACK = op
    return op


def build_nc(use_b1=True, use_b2=True, use_db1=True, use_db2=True):
    qpack = _register_qpack()
    nc = bacc.Bacc(
        "TRN2",
        target_bir_lowering=False,
        debug=False,
        enable_asserts=False,
        num_devices=NCORES,
    )
    Relu = mybir.ActivationFunctionType.Relu
    Square = mybir.ActivationFunctionType.Square
    Sqrt = mybir.ActivationFunctionType.Sqrt

    # ---------------- DRAM I/O ----------------
    x_d = nc.dram_tensor("xbt", [OBS, NSH], bf16, kind="ExternalInput")
    w1_d = nc.dram_tensor("w1b", [OBS, HID], bf16, kind="ExternalInput")
    b1_d = nc.dram_tensor("b1", [HID // 512, 512], bf16, kind="ExternalInput")
    w2_d = nc.dram_tensor("w2b", [HID, LAT], bf16, kind="ExternalInput")
    b2_d = nc.dram_tensor("b2", [1, LAT], bf16, kind="ExternalInput")
    dw1_d = nc.dram_tensor("dw1b", [LAT, HID], bf16, kind="ExternalInput")
    db1_d = nc.dram_tensor("db1", [HID // 512, 512], bf16, kind="ExternalInput")
    dw2_d = nc.dram_tensor("dw2b", [HID, OBS], bf16, kind="ExternalInput")
    db2_d = nc.dram_tensor("db2", [OBS // 512, 512], bf16, kind="ExternalInput")
    e2t_d = nc.dram_tensor("e2t", [HQ, LAT, VOCAB], bf16, kind="ExternalInput")
    se2_d = nc.dram_tensor("se2", [HQ, 128, VOCAB], bf16, kind="ExternalInput")
    e2tp_d = nc.dram_tensor("e2tp", [HQ, 128, VOCAB], mybir.dt.uint32, kind="ExternalInput")
    identb_d = nc.dram_tensor("identb", [128, 128], bf16, kind="ExternalInput")
    d_giota = nc.dram_tensor("giota", [128, RT * 4], f32, kind="ExternalInput")
    out_d = nc.dram_tensor("out", [128, 40], f32, kind="ExternalOutput")

    import contextlib

    with tile.TileContext(nc) as tc, contextlib.ExitStack() as ctx:
        const_p = ctx.enter_context(tc.tile_pool(name="const", bufs=1))
        persist_p = ctx.enter_context(tc.tile_pool(name="persist", bufs=1))
        small_p = ctx.enter_context(tc.tile_pool(name="small", bufs=4))

        # ---- constants ----
        identb = const_p.tile([128, 128], bf16, name="identb")
        nc.sync.dma_start(identb[:], identb_d.ap())
        ones1 = const_p.tile([1, 512], bf16, name="ones1")
        nc.vector.memset(ones1[:], 1.0)
        out_sb = const_p.tile([128, 40], f32, name="out_sb")
        nc.vector.memset(out_sb[:], 0.0)
        epsc = const_p.tile([128, 1], f32, name="epsc")
        nc.vector.memset(epsc[:], LN_EPS)

        # ---- persistent ----
        latT = persist_p.tile([128, 2, NSH], f32, name="latT")
        residT = persist_p.tile([128, 2, NSH], f32, name="residT")
        residTb = persist_p.tile([128, 2, NSH], bf16, name="residTb")
        trash = persist_p.tile([128, 2048], bf16, name="trash")
        idx16 = persist_p.tile([128, RT], i16, name="idx16")
        idxg = persist_p.tile([128, RT, 8], i16, name="idxg")
        nc.vector.memset(idxg[:], 0)
        # =============== encoder ===============
        enc_ctx = contextlib.ExitStack()
        enc_p = enc_ctx.enter_context(tc.tile_pool(name="encp", bufs=1))
        work_p = enc_ctx.enter_context(tc.tile_pool(name="encw", bufs=2))
        eps_h = enc_ctx.enter_context(tc.tile_pool(name="epsh", bufs=6, space="PSUM"))
        eps_t = enc_ctx.enter_context(tc.tile_pool(name="epst", bufs=2, space="PSUM"))
        w2s = enc_p.tile([128, HID // 128, LAT], bf16, name="w2s")
        w1s = enc_p.tile([128, OBS // 128, HID], bf16, name="w1s")
        for k in range(OBS // 128):
            nc.sync.dma_start(w1s[:, k, :], w1_d.ap()[k * 128:(k + 1) * 128, :])
        for k in range(HID // 128):
            nc.sync.dma_start(w2s[:, k, :], w2_d.ap()[k * 128:(k + 1) * 128, :])
        b1s = const_p.tile([HID // 512, 512], bf16, name="b1s")
        if use_b1:
            nc.sync.dma_start(b1s[:], b1_d.ap())
        b2s = const_p.tile([1, LAT], bf16, name="b2s")
        if use_b2:
            nc.sync.dma_start(b2s[:], b2_d.ap())

        xT_sb = enc_p.tile([128, OBS // 128, NSH], bf16, name="xT_sb")
        for k in range(OBS // 128):
            nc.sync.dma_start(xT_sb[:, k, :], x_d.ap()[k * 128:(k + 1) * 128, :])
        for rc in range(8):  # row chunks of 256 (2 row tiles each)
            hTc = work_p.tile([128, HID // 128, 256], bf16, name="hTc", tag="hT")
            for rj in range(2):
                rt = rc * 2 + rj
                # h = x @ W1 (+ b1), chunk-grained PSUM for overlap
                hsb = work_p.tile([128, 2048], f32, name="hsb", tag="hsb")
                bns = small_p.tile([128, 4, 6], f32, name="bns", tag="s1")
                for cc in range(4):
                    hps = eps_h.tile([128, 512], f32, name="hps", tag="hps")
                    nmm = OBS // 128
                    for k in range(nmm):
                        nc.tensor.matmul(
                            hps[:],
                            xT_sb[:, k, rt * 128:(rt + 1) * 128],
                            w1s[:, k, cc * 512:(cc + 1) * 512],
                            start=(k == 0),
                            stop=(k == nmm - 1 and not use_b1),
                        )
                    if use_b1:
                        nc.tensor.matmul(
                            hps[:],
                            ones1[:, 0:128],
                            b1s[cc:cc + 1, :],
                            start=False, stop=True,
                        )
                    nc.scalar.activation(
                        hsb[:, cc * 512:(cc + 1) * 512], hps[:],
                        mybir.ActivationFunctionType.Copy,
                    )
                    nc.vector.bn_stats(bns[:, cc, :], hps[:])
                mv = small_p.tile([128, 2], f32, name="mv", tag="s2")
                nc.vector.bn_aggr(mv[:], bns[:].rearrange("p a b -> p (a b)"))
                std = small_p.tile([128, 1], f32, name="std", tag="s6")
                nc.scalar.activation(std[:], mv[:, 1:2], Sqrt, bias=epsc[:])
                rstd = small_p.tile([128, 1], f32, name="rstd", tag="s7")
                nc.vector.reciprocal(rstd[:], std[:])
                nmr = small_p.tile([128, 1], f32, name="nmr", tag="s8")
                nc.vector.tensor_scalar(
                    nmr[:], mv[:, 0:1], rstd[:], -1.0,
                    op0=AluOpType.mult, op1=AluOpType.mult,
                )
                hrelu = work_p.tile([128, 2048], bf16, name="hrelu", tag="hrelu")
                nc.scalar.activation(
                    hrelu[:], hsb[:], Relu, bias=nmr[:], scale=rstd[:]
                )
                for o in range(HID // 128):
                    nc.sync.dma_start_transpose(
                        hTc[:, o, rj * 128:(rj + 1) * 128],
                        hrelu[:, o * 128:(o + 1) * 128],
                    )
            # latent^T for these 256 rows
            for m in range(2):
                lps = eps_h.tile([128, 256], f32, name="lps", tag="hps")
                nk = HID // 128
                for k in range(nk):
                    nc.tensor.matmul(
                        lps[:, 0:256],
                        w2s[:, k, m * 128:(m + 1) * 128],
                        hTc[:, k, :],
                        start=(k == 0),
                        stop=(k == nk - 1 and not use_b2),
                    )
                if use_b2:
                    nc.tensor.matmul(
                        lps[:, 0:256],
                        b2s[:, m * 128:(m + 1) * 128],
                        ones1[:, 0:256],
                        start=False, stop=True,
                    )
                nc.vector.tensor_copy(
                    latT[:, m, rc * 256:(rc + 1) * 256], lps[:, 0:256]
                )

        for m in range(2):
            nc.vector.tensor_copy(residT[:, m, :], latT[:, m, :])
            nc.vector.tensor_copy(residTb[:, m, :], latT[:, m, :])

        enc_ctx.close()

        # =============== RVQ ===============
        vq_ctx = contextlib.ExitStack()
        vq_p = vq_ctx.enter_context(tc.tile_pool(name="vqp", bufs=1))
        vps_p = vq_ctx.enter_context(tc.tile_pool(name="vps", bufs=2, space="PSUM"))
        e2ts = vq_p.tile([128, 2, 2, VOCAB], bf16, name="e2ts")
        e2tp = vq_p.tile([128, VOCAB], mybir.dt.uint32, name="e2tp")
        qp = vq_p.tile([128, NSH], mybir.dt.uint32, name="qp")
        se2b = vq_p.tile([128, 2, VOCAB], bf16, name="se2b")
        from concourse import library_config
        nc.gpsimd.load_library(library_config.ap_gather)
        pk64 = persist_p.tile([128, RT * 4], f32, name="pk64")
        giota = const_p.tile([128, RT * 4], f32, name="giota")
        nc.sync.dma_start(giota[:], d_giota.ap())
        for lv in range(HQ):
            db = lv % 2
            for m in range(2):
                nc.sync.dma_start(
                    e2ts[:, db, m, :], e2t_d.ap()[lv, m * 128:(m + 1) * 128, :]
                )
            nc.sync.dma_start(se2b[:, db, :], se2_d.ap()[lv])
            nc.sync.dma_start(e2tp[:], e2tp_d.ap()[lv])
            qb = qp[:].bitcast(bf16).rearrange("p (n two) -> p n two", two=2)
            for qq in range(4):  # quarters of 4 row tiles, pipelined
                for rj in range(4):
                    rt = qq * 4 + rj
                    for g in range(4):
                        sps = vps_p.tile([128, 2048], f32, name="sps", tag="sps")
                        for cc in range(4):
                            c0 = g * 2048 + cc * 512
                            for k in range(2):
                                nc.tensor.matmul(
                                    sps[:, cc * 512:(cc + 1) * 512],
                                    residTb[:, k, rt * 128:(rt + 1) * 128],
                                    e2ts[:, db, k, c0:c0 + 512],
                                    start=(k == 0), stop=(k == 1),
                                )
                        nc.vector._custom_dve(
                            qpack,
                            out=trash[:],
                            in0=sps[:],
                            in1=se2b[:, db, g * 2048:(g + 1) * 2048],
                            imm2=BIG,
                            accum_out=pk64[:, rt * 4 + g: rt * 4 + g + 1],
                        )
                # index extraction for this quarter
                cs = qq * 16
                pk3 = pk64[:, cs:cs + 16].rearrange("p (a b) -> p a b", a=4)
                m16 = small_p.tile([128, 4], f32, name="m16", tag="m16")
                nc.vector.tensor_reduce(
                    m16[:], pk3, axis=mybir.AxisListType.X, op=AluOpType.max
                )
                msk = small_p.tile([128, 4, 4], f32, name="msk", tag="msk")
                nc.vector.tensor_tensor(
                    msk[:], pk3,
                    m16[:].rearrange("p (a o) -> p a o", o=1)
                    .broadcast_to((128, 4, 4)),
                    op=AluOpType.is_ge,
                )
                nc.vector.tensor_mul(
                    msk[:], msk[:],
                    giota[:, cs:cs + 16].rearrange("p (a b) -> p a b", a=4),
                )
                gidx = small_p.tile([128, 4], f32, name="gidx", tag="gidx")
                nc.vector.tensor_reduce(
                    gidx[:], msk[:], axis=mybir.AxisListType.X, op=AluOpType.add
                )
                nc.vector.tensor_scalar_min(gidx[:], gidx[:], 3.0)
                mi = small_p.tile([128, 4], mybir.dt.int32, name="mi", tag="mi")
                nc.vector.tensor_copy(mi[:], m16[:])
                nc.vector.tensor_scalar(
                    mi[:], mi[:], int(GRID) - 1, None, op0=AluOpType.bitwise_and
                )
                loc = small_p.tile([128, 4], f32, name="loc", tag="loc")
                nc.vector.tensor_copy(loc[:], mi[:])
                nc.vector.tensor_scalar(
                    gidx[:], gidx[:], GRID, None, op0=AluOpType.mult
                )
                nc.vector.tensor_add(loc[:], loc[:], gidx[:])
                nc.vector.tensor_copy(idx16[:, qq * 4:(qq + 1) * 4], loc[:])
                # stage indices (wrapped + replicated across 8 Q7 groups)
                for kk in range(8):
                    nc.gpsimd.dma_start(
                        idxg[0:16, qq * 4:(qq + 1) * 4, kk],
                        idx16[kk * 16:(kk + 1) * 16, qq * 4:(qq + 1) * 4],
                    )
                for gg in range(1, 8):
                    nc.gpsimd.dma_start(
                        idxg[gg * 16:(gg + 1) * 16, qq * 4:(qq + 1) * 4, :],
                        idxg[0:16, qq * 4:(qq + 1) * 4, :],
                    )
                nc.gpsimd.ap_gather(
                    qp[:, qq * 512:(qq + 1) * 512],
                    e2tp[:],
                    idxg[:, qq * 4:(qq + 1) * 4, :].rearrange("p a b -> p (a b)"),
                    channels=128, num_elems=VOCAB, d=1, num_idxs=512,
                )
                # apply the PREVIOUS quarter's update here so the in-order
                # DVE never waits on this quarter's gather
                for uq in ([qq - 1] if qq > 0 else []) + ([qq] if qq == 3 else []):
                    for m in range(2):
                        nc.vector.tensor_sub(
                            residT[:, m, uq * 512:(uq + 1) * 512]
                            .rearrange("p (n o) -> p n o", o=1),
                            residT[:, m, uq * 512:(uq + 1) * 512]
                            .rearrange("p (n o) -> p n o", o=1),
                            qb[:, uq * 512:(uq + 1) * 512, m:m + 1],
                        )
                        if lv < HQ - 1:
                            nc.vector.tensor_copy(
                                residTb[:, m, uq * 512:(uq + 1) * 512],
                                residT[:, m, uq * 512:(uq + 1) * 512],
                            )
            for m in range(2):
                nc.scalar.activation(
                    trash[:], residT[:, m, :], Square,
                    accum_out=out_sb[:, 2 * lv + m: 2 * lv + m + 1],
                )

        for m in range(2):
            # quant^T = latT - residT (stored back into latT)
            nc.vector.tensor_sub(latT[:, m, :], latT[:, m, :], residT[:, m, :])

        vq_ctx.close()

        # =============== decoder ===============
        dec_ctx = contextlib.ExitStack()
        dec_p = dec_ctx.enter_context(tc.tile_pool(name="decp", bufs=1))
        work_p = dec_ctx.enter_context(tc.tile_pool(name="decw", bufs=2))
        dps_p = dec_ctx.enter_context(tc.tile_pool(name="dps", bufs=4, space="PSUM"))
        dw1s = dec_p.tile([128, 2, HID], bf16, name="dw1s")
        quantTb = dec_p.tile([128, 2, NSH], bf16, name="quantTb")
        for m in range(2):
            nc.vector.tensor_copy(quantTb[:, m, :], latT[:, m, :])
        for k in range(2):
            nc.sync.dma_start(dw1s[:, k, :], dw1_d.ap()[k * 128:(k + 1) * 128, :])
        dw2s = dec_p.tile([128, HID // 128, OBS], bf16, name="dw2s")
        for k in range(HID // 128):
            nc.sync.dma_start(dw2s[:, k, :], dw2_d.ap()[k * 128:(k + 1) * 128, :])
        db1s = const_p.tile([HID // 512, 512], bf16, name="db1s")
        if use_db1:
            nc.sync.dma_start(db1s[:], db1_d.ap())
        db2s = const_p.tile([OBS // 512, 512], bf16, name="db2s")
        if use_db2:
            nc.sync.dma_start(db2s[:], db2_d.ap())

        for rc in range(4):  # row chunks of 512
            dhT = work_p.tile([128, HID // 128, 512], bf16, name="dhT", tag="hT")
            for ht in range(HID // 128):
                dps = dps_p.tile([128, 512], f32, name="dps", tag="dmm")
                for k in range(2):
                    nc.tensor.matmul(
                        dps[:, 0:512],
                        dw1s[:, k, ht * 128:(ht + 1) * 128],
                        quantTb[:, k, rc * 512:(rc + 1) * 512],
                        start=(k == 0), stop=(k == 1 and not use_db1),
                    )
                if use_db1:
                    nc.tensor.matmul(
                        dps[:, 0:512],
                        db1s[(ht * 128) // 512:(ht * 128) // 512 + 1,
                             (ht * 128) % 512:(ht * 128) % 512 + 128],
                        ones1[:],
                        start=False, stop=True,
                    )
                nc.scalar.activation(dhT[:, ht, :], dps[:, 0:512], Relu)
            for ot in range(OBS // 128):
                xTl = work_p.tile([128, 512], bf16, name="xTl", tag="xTl")
                nc.sync.dma_start(
                    xTl[:],
                    x_d.ap()[ot * 128:(ot + 1) * 128, rc * 512:(rc + 1) * 512],
                )
                rps = dps_p.tile([128, 512], f32, name="rps", tag="dmm")
                nk = HID // 128
                for k in range(nk):
                    nc.tensor.matmul(
                        rps[:, 0:512],
                        dw2s[:, k, ot * 128:(ot + 1) * 128],
                        dhT[:, k, :],
                        start=(k == 0), stop=(k == nk - 1 and not use_db2),
                    )
                if use_db2:
                    nc.tensor.matmul(
                        rps[:, 0:512],
                        db2s[(ot * 128) // 512:(ot * 128) // 512 + 1,
                             (ot * 128) % 512:(ot * 128) % 512 + 128],
                        ones1[:],
                        start=False, stop=True,
                    )
                diff = work_p.tile([128, 512], f32, name="diff", tag="diff")
                nc.vector.tensor_sub(diff[:], rps[:, 0:512], xTl[:])
                nc.scalar.activation(
                    diff[:], diff[:], Square,
                    accum_out=out_sb[:, 8 + rc * 8 + ot: 9 + rc * 8 + ot],
                )

        dec_ctx.close()
        nc.sync.dma_start(out_d.ap(), out_sb[:])

    nc.compile()
    return nc


def _host_prep(inputs):
    import ml_dtypes

    x = np.asarray(inputs["x"], np.float32)
    cb = np.ascontiguousarray(np.asarray(inputs["codebooks"], np.float32))
    w1 = np.ascontiguousarray(np.asarray(inputs["enc_w1"], np.float32))
    b1 = np.asarray(inputs["enc_b1"], np.float32)
    lng = np.asarray(inputs["ln_g"], np.float32)
    lnb = np.asarray(inputs["ln_b"], np.float32)
    w2 = np.asarray(inputs["enc_w2"], np.float32)
    b2 = np.asarray(inputs["enc_b2"], np.float32)
    dw1 = np.ascontiguousarray(np.asarray(inputs["dec_w1"], np.float32))
    db1 = np.asarray(inputs["dec_b1"], np.float32)
    dw2 = np.asarray(inputs["dec_w2"], np.float32)
    db2 = np.asarray(inputs["dec_b2"], np.float32)

    assert np.all(lnb == 0.0) and np.all(lng > 0.0), "kernel assumes ln_b==0, ln_g>0"
    w2g = w2 * lng[:, None]  # relu(z*g)@W2 == relu(z)@(g[:,None]*W2) for g>0

    # sample-estimate per-level score ranges to pick K, SHIFT
    rng = np.random.default_rng(0)
    sel = rng.choice(x.shape[0], 256, replace=False)
    h = x[sel] @ w1 + b1
    mu = h.mean(-1, keepdims=True)
    var = ((h - mu) ** 2).mean(-1, keepdims=True)
    h = np.maximum((h - mu) / np.sqrt(var + LN_EPS) * lng + lnb, 0.0)
    resid = h @ w2 + b2
    e2sum = (cb.astype(np.float64) ** 2).sum(-1).astype(np.float32)  # [HQ, VOCAB]
    Ks, SHIFTs = [], []
    for lv in range(HQ):
        sc = 2.0 * resid @ cb[lv].T - e2sum[lv]
        lo, hi = float(sc.min()), float(sc.max())
        span = hi - lo
        shift = -lo + 0.75 * span + 16.0       # margin: scores stay well positive
        smax = (hi + shift) * 2.0              # 2x safety for sample underestimate
        K = np.float32((2.0**24 * 0.98) / smax)
        Ks.append(K)
        SHIFTs.append(np.float32(shift))
        idx = sc.argmax(-1)
        resid = resid - cb[lv][idx]

    e2t = cb.transpose(0, 2, 1)  # [HQ, LAT, VOCAB]
    e2t_bf = np.zeros((HQ, LAT, VOCAB), ml_dtypes.bfloat16)
    se2 = np.zeros((HQ, 128, VOCAB), ml_dtypes.bfloat16)
    e2tp_pack = np.zeros((HQ, 128, VOCAB), np.uint32)
    for lv in range(HQ):
        e2t_bf[lv] = (np.float32(2.0 * Ks[lv]) * e2t[lv]).astype(ml_dtypes.bfloat16)
        row = (Ks[lv] * (SHIFTs[lv] - e2sum[lv])).astype(ml_dtypes.bfloat16)
        se2[lv] = np.broadcast_to(row, (128, VOCAB))
        pk0 = e2t[lv, :128].astype(ml_dtypes.bfloat16).view(np.uint16).astype(np.uint32)
        pk1 = e2t[lv, 128:].astype(ml_dtypes.bfloat16).view(np.uint16).astype(np.uint32)
        e2tp_pack[lv] = pk0 | (pk1 << 16)

    common = {
        "w1b": np.ascontiguousarray(w1.astype(ml_dtypes.bfloat16)),
        "b1": np.ascontiguousarray(b1.reshape(HID // 512, 512).astype(ml_dtypes.bfloat16)),
        "w2b": np.ascontiguousarray(w2g.astype(ml_dtypes.bfloat16)),
        "b2": b2.reshape(1, LAT).astype(ml_dtypes.bfloat16),
        "dw1b": np.ascontiguousarray(dw1.astype(ml_dtypes.bfloat16)),
        "db1": np.ascontiguousarray(db1.reshape(HID // 512, 512).astype(ml_dtypes.bfloat16)),
        "dw2b": np.ascontiguousarray(dw2.astype(ml_dtypes.bfloat16)),
        "db2": np.ascontiguousarray(db2.reshape(OBS // 512, 512).astype(ml_dtypes.bfloat16)),
        "e2t": np.ascontiguousarray(e2t_bf),
        "se2": np.ascontiguousarray(se2),
        "e2tp": e2tp_pack,
        "identb": np.eye(128, dtype=np.float32).astype(ml_dtypes.bfloat16),
        "giota": np.ascontiguousarray(
            np.tile(np.arange(4, dtype=np.float32), (128, RT))
        ),
    }
    flags = dict(
        use_b1=bool(np.any(b1 != 0)),
        use_b2=bool(np.any(b2 != 0)),
        use_db1=bool(np.any(db1 != 0)),
        use_db2=bool(np.any(db2 != 0)),
    )
    in_maps = []
    for c in range(NCORES):
        m = dict(common)
        m["xbt"] = np.ascontiguousarray(
            x[c * NSH:(c + 1) * NSH].T.astype(ml_dtypes.bfloat16)
        )
        in_maps.append(m)
    return in_maps, flags


def _combine(results):
    rlv = rrec = 0.0
    for c in range(NCORES):
        o = np.asarray(results[c]["out"], np.float64)
        rlv += o[:, 0:8].sum()
        rrec += o[:, 8:40].sum()
    return np.float32(1.5 * rlv / (N * LAT) + 0.5 * rrec / (N * OBS))


_NC_CACHE = {}


def get_nc(flags):
    key = tuple(sorted(flags.items()))
    if key not in _NC_CACHE:
        _NC_CACHE[key] = build_nc(**flags)
    return _NC_CACHE[key]


def kernel(**inputs) -> np.ndarray:
    in_maps, flags = _host_prep(inputs)
    nc = get_nc(flags)
    res = run_bass_kernel_spmd(nc, in_maps, core_ids=list(range(NCORES)))
    return _combine(res.results)


# revision 28
# speedup vs baseline: 1.1837x; 1.1837x over previous
# Trainium2 Bass kernel for residual-VQ autoencoder loss (vq_codebook).
# Data-parallel over rows: 8 NeuronCores, 2048 rows each; codebooks/weights
# replicated. The scalar loss is assembled on host from per-core [128,40]
# partial-sum outputs (no on-chip collectives needed).
#
# Per core (RT=16 row-tiles of 128):
#   encoder: h = x@W1+b1 -> LN -> ReLU -> latent = h@W2' + b2 (ln_g folded into W2)
#   RVQ: psum_score[r,v] = 2K * r . E_v  (bf16 matmul, pre-scaled E^T)
#     one custom-DVE pass per [128,2048] PSUM group adds K*(SHIFT-|E_v|^2),
#     quantizes via (x+BIG)-BIG to multiples of 8192, adds the global vocab
#     index, and MAX-accumulates (chained across 4 groups) => packed argmax.
#     idx = packed mod 8192; q = codebook[idx] via gpsimd.dma_gather (exact f32);
#     resid -= q^T (PE transpose + DVE sub).
#   loss telescopes: sum_l 1.5*mean((q_l-r_l)^2) = 1.5*(R0-R4)/(N*LAT);
#   quant^T = latent^T - resid^T; decoder stays feature-major (no transposes);
#   Rrec = sum((recon-x)^2) using x^T spilled to DRAM during the encoder.

import sys

sys.path.insert(0, "/opt/trn_rl_repo")

import numpy as np

import concourse.bass as bass
import concourse.mybir as mybir
import concourse.bacc as bacc
import concourse.tile as tile
from concourse.alu_op_type import AluOpType
from concourse.bass_utils import run_bass_kernel_spmd

OBS, HID, LAT = 1024, 2048, 256
VOCAB, HQ = 8192, 4
N, NCORES = 16384, 8
NSH = N // NCORES          # 2048 rows per core
RT = NSH // 128            # 16 row tiles
LN_EPS = 1e-5
GRID = 2048.0               # one vocab group per QPACK call; 11 index bits
BIG = float(1.5 * 2.0**34)  # ulp(BIG)=2048 -> (x+BIG)-BIG rounds to mult of GRID

f32 = mybir.dt.float32
f32r = mybir.dt.float32r
bf16 = mybir.dt.bfloat16
i16 = mybir.dt.int16

_QPACK = None


def _register_qpack():
    """out = (((Src0+Src1)+BIG)-BIG) + Idx; accum_out = max over free dim.
    Src0: PSUM scores 2K*r.E (f32); Src1: K*(SHIFT-|E|^2) replicated (bf16).
    Quantizes the score to multiples of GRID and packs the within-group index
    into the low bits; per-group accums are combined at level end."""
    global _QPACK
    if _QPACK is not None:
        return _QPACK
    from concourse import dve_ops
    from concourse.dve_spec import Spec, Src0, Src1, C2, AluOp, Idx, lower
    from concourse.dve_table_gen import DveOpSpec

    def _ref(in0, in1, s0, s1, imm2):
        x = np.asarray(in0, np.float32) + np.asarray(in1, np.float32)
        q = (x + np.float32(imm2)).astype(np.float32) - np.float32(imm2)
        idxv = np.arange(x.shape[-1], dtype=np.float32)
        out = (q + idxv).astype(np.float32)
        acc = out.max(axis=-1, keepdims=True)
        return out, acc

    body = (((Src0 + Src1) + C2) - C2) + Idx
    spec = Spec(body=body, accum=AluOp.MAX, reference=_ref)
    op = dve_ops.DveOp("QPACK_ARGMAX", spec, subdim=False, uops_sha={})
    dve_ops.OPS.append(op)
    dve_ops.CUSTOM_DVE_SPECS[op.name] = op.spec
    dve_ops._SUB_OPCODE_FOR_NAME[op.name] = (
        dve_ops._CUSTOM_DVE_ROW_BASE + len(dve_ops.OPS) - 1
    )
    for ver in ("v3", "v4"):
        s = DveOpSpec(
            name=op.name,
            opcode=dve_ops.get_dve_sub_opcode(op.name),
            uops=lower(spec, ver=ver),
            rd1_en=True,
        )
        op.uops_sha[ver] = s.sha(ver)
    _QPACK = op
    return op


def build_nc(use_b1=True, use_b2=True, use_db1=True, use_db2=True):
    qpack = _register_qpack()
    nc = bacc.Bacc(
        "TRN2",
        target_bir_lowering=False,
        debug=False,
        enable_asserts=False,
        num_devices=NCORES,
    )
    Relu = mybir.ActivationFunctionType.Relu
    Square = mybir.ActivationFunctionType.Square
    Sqrt = mybir.ActivationFunctionType.Sqrt

    # ---------------- DRAM I/O ----------------
    x_d = nc.dram_tensor("xbt", [OBS, NSH], bf16, kind="ExternalInput")
    w1_d = nc.dram_tensor("w1b", [OBS, HID], bf16, kind="ExternalInput")
    b1_d = nc.dram_tensor("b1", [HID // 512, 512], bf16, kind="ExternalInput")
    w2_d = nc.dram_tensor("w2b", [HID, LAT], bf16, kind="ExternalInput")
    b2_d = nc.dram_tensor("b2", [1, LAT], bf16, kind="ExternalInput")
    dw1_d = nc.dram_tensor("dw1b", [LAT, HID], bf16, kind="ExternalInput")
    db1_d = nc.dram_tensor("db1", [HID // 512, 512], bf16, kind="ExternalInput")
    dw2_d = nc.dram_tensor("dw2b", [HID, OBS], bf16, kind="ExternalInput")
    db2_d = nc.dram_tensor("db2", [OBS // 512, 512], bf16, kind="ExternalInput")
    e2t_d = nc.dram_tensor("e2t", [HQ, LAT, VOCAB], bf16, kind="ExternalInput")
    se2_d = nc.dram_tensor("se2", [HQ, 128, VOCAB], bf16, kind="ExternalInput")
    e2tp_d = nc.dram_tensor("e2tp", [HQ, 128, VOCAB], mybir.dt.uint32, kind="ExternalInput")
    identb_d = nc.dram_tensor("identb", [128, 128], bf16, kind="ExternalInput")
    d_giota = nc.dram_tensor("giota", [128, RT * 4], f32, kind="ExternalInput")
    out_d = nc.dram_tensor("out", [128, 40], f32, kind="ExternalOutput")

    import contextlib

    with tile.TileContext(nc) as tc, contextlib.ExitStack() as ctx:
        const_p = ctx.enter_context(tc.tile_pool(name="const", bufs=1))
        persist_p = ctx.enter_context(tc.tile_pool(name="persist", bufs=1))
        small_p = ctx.enter_context(tc.tile_pool(name="small", bufs=4))

        # ---- constants ----
        identb = const_p.tile([128, 128], bf16, name="identb")
        nc.sync.dma_start(identb[:], identb_d.ap())
        ones1 = const_p.tile([1, 512], bf16, name="ones1")
        nc.vector.memset(ones1[:], 1.0)
        out_sb = const_p.tile([128, 40], f32, name="out_sb")
        nc.vector.memset(out_sb[:], 0.0)
        epsc = const_p.tile([128, 1], f32, name="epsc")
        nc.vector.memset(epsc[:], LN_EPS)

        # ---- persistent ----
        latT = persist_p.tile([128, 2, NSH], f32, name="latT")
        residT = persist_p.tile([128, 2, NSH], f32, name="residT")
        residTb = persist_p.tile([128, 2, NSH], bf16, name="residTb")
        trash = persist_p.tile([128, 2048], bf16, name="trash")
        idx16 = persist_p.tile([128, RT], i16, name="idx16")
        idxg = persist_p.tile([128, RT, 8], i16, name="idxg")
        nc.vector.memset(idxg[:], 0)
        # =============== encoder ===============
        enc_ctx = contextlib.ExitStack()
        enc_p = enc_ctx.enter_context(tc.tile_pool(name="encp", bufs=1))
        work_p = enc_ctx.enter_context(tc.tile_pool(name="encw", bufs=2))
        eps_h = enc_ctx.enter_context(tc.tile_pool(name="epsh", bufs=6, space="PSUM"))
        eps_t = enc_ctx.enter_context(tc.tile_pool(name="epst", bufs=2, space="PSUM"))
        w2s = enc_p.tile([128, HID // 128, LAT], bf16, name="w2s")
        w1s = enc_p.tile([128, OBS // 128, HID], bf16, name="w1s")
        for k in range(OBS // 128):
            nc.sync.dma_start(w1s[:, k, :], w1_d.ap()[k * 128:(k + 1) * 128, :])
        for k in range(HID // 128):
            nc.sync.dma_start(w2s[:, k, :], w2_d.ap()[k * 128:(k + 1) * 128, :])
        b1s = const_p.tile([HID // 512, 512], bf16, name="b1s")
        if use_b1:
            nc.sync.dma_start(b1s[:], b1_d.ap())
        b2s = const_p.tile([1, LAT], bf16, name="b2s")
        if use_b2:
            nc.sync.dma_start(b2s[:], b2_d.ap())

        xT_sb = enc_p.tile([128, OBS // 128, NSH], bf16, name="xT_sb")
        for k in range(OBS // 128):
            nc.sync.dma_start(xT_sb[:, k, :], x_d.ap()[k * 128:(k + 1) * 128, :])
        for rc in range(8):  # row chunks of 256 (2 row tiles each)
            hTc = work_p.tile([128, HID // 128, 256], bf16, name="hTc", tag="hT")
            for rj in range(2):
                rt = rc * 2 + rj
                # h = x @ W1 (+ b1), chunk-grained PSUM for overlap
                hsb = work_p.tile([128, 2048], f32, name="hsb", tag="hsb")
                bns = small_p.tile([128, 4, 6], f32, name="bns", tag="s1")
                for cc in range(4):
                    hps = eps_h.tile([128, 512], f32, name="hps", tag="hps")
                    nmm = OBS // 128
                    for k in range(nmm):
                        nc.tensor.matmul(
                            hps[:],
                            xT_sb[:, k, rt * 128:(rt + 1) * 128],
                            w1s[:, k, cc * 512:(cc + 1) * 512],
                            start=(k == 0),
                            stop=(k == nmm - 1 and not use_b1),
                        )
                    if use_b1:
                        nc.tensor.matmul(
                            hps[:],
                            ones1[:, 0:128],
                            b1s[cc:cc + 1, :],
                            start=False, stop=True,
                        )
                    nc.scalar.activation(
                        hsb[:, cc * 512:(cc + 1) * 512], hps[:],
                        mybir.ActivationFunctionType.Copy,
                    )
                    nc.vector.bn_stats(bns[:, cc, :], hps[:])
                mv = small_p.tile([128, 2], f32, name="mv", tag="s2")
                nc.vector.bn_aggr(mv[:], bns[:].rearrange("p a b -> p (a b)"))
                std = small_p.tile([128, 1], f32, name="std", tag="s6")
                nc.scalar.activation(std[:], mv[:, 1:2], Sqrt, bias=epsc[:])
                rstd = small_p.tile([128, 1], f32, name="rstd", tag="s7")
                nc.vector.reciprocal(rstd[:], std[:])
                nmr = small_p.tile([128, 1], f32, name="nmr", tag="s8")
                nc.vector.tensor_scalar(
                    nmr[:], mv[:, 0:1], rstd[:], -1.0,
                    op0=AluOpType.mult, op1=AluOpType.mult,
                )
                hrelu = work_p.tile([128, 2048], bf16, name="hrelu", tag="hrelu")
                nc.scalar.activation(
                    hrelu[:], hsb[:], Relu, bias=nmr[:], scale=rstd[:]
                )
                for o in range(HID // 128):
                    htp = eps_t.tile([128, 128], f32, name="htp", tag="tp")
                    nc.tensor.matmul(
                        htp[:, 0:64].bitcast(bf16),
                        hrelu[:, o * 128:(o + 1) * 128],
                        identb[:], is_transpose=True, start=True, stop=True,
                    )
                    nc.vector.tensor_copy(
                        hTc[:, o, rj * 128:(rj + 1) * 128], htp[:, 0:64].bitcast(bf16)
                    )
            # latent^T for these 256 rows
            for m in range(2):
                lps = eps_h.tile([128, 256], f32, name="lps", tag="hps")
                nk = HID // 128
                for k in range(nk):
                    nc.tensor.matmul(
                        lps[:, 0:256],
                        w2s[:, k, m * 128:(m + 1) * 128],
                        hTc[:, k, :],
                        start=(k == 0),
                        stop=(k == nk - 1 and not use_b2),
                    )
                if use_b2:
                    nc.tensor.matmul(
                        lps[:, 0:256],
                        b2s[:, m * 128:(m + 1) * 128],
                        ones1[:, 0:256],
                        start=False, stop=True,
                    )
                nc.vector.tensor_copy(
                    latT[:, m, rc * 256:(rc + 1) * 256], lps[:, 0:256]
                )

        for m in range(2):
            nc.vector.tensor_copy(residT[:, m, :], latT[:, m, :])
            nc.vector.tensor_copy(residTb[:, m, :], latT[:, m, :])

        enc_ctx.close()

        # =============== RVQ ===============
        vq_ctx = contextlib.ExitStack()
        vq_p = vq_ctx.enter_context(tc.tile_pool(name="vqp", bufs=1))
        vps_p = vq_ctx.enter_context(tc.tile_pool(name="vps", bufs=2, space="PSUM"))
        e2ts = vq_p.tile([128, 2, 2, VOCAB], bf16, name="e2ts")
        e2tp = vq_p.tile([128, VOCAB], mybir.dt.uint32, name="e2tp")
        qp = vq_p.tile([128, NSH], mybir.dt.uint32, name="qp")
        se2b = vq_p.tile([128, 2, VOCAB], bf16, name="se2b")
        from concourse import library_config
        nc.gpsimd.load_library(library_config.ap_gather)
        pk64 = persist_p.tile([128, RT * 4], f32, name="pk64")
        giota = const_p.tile([128, RT * 4], f32, name="giota")
        nc.sync.dma_start(giota[:], d_giota.ap())
        for lv in range(HQ):
            db = lv % 2
            for m in range(2):
                nc.sync.dma_start(
                    e2ts[:, db, m, :], e2t_d.ap()[lv, m * 128:(m + 1) * 128, :]
                )
            nc.sync.dma_start(se2b[:, db, :], se2_d.ap()[lv])
            nc.sync.dma_start(e2tp[:], e2tp_d.ap()[lv])
            qb = qp[:].bitcast(bf16).rearrange("p (n two) -> p n two", two=2)
            for qq in range(4):  # quarters of 4 row tiles, pipelined
                for rj in range(4):
                    rt = qq * 4 + rj
                    for g in range(4):
                        sps = vps_p.tile([128, 2048], f32, name="sps", tag="sps")
                        for cc in range(4):
                            c0 = g * 2048 + cc * 512
                            for k in range(2):
                                nc.tensor.matmul(
                                    sps[:, cc * 512:(cc + 1) * 512],
                                    residTb[:, k, rt * 128:(rt + 1) * 128],
                                    e2ts[:, db, k, c0:c0 + 512],
                                    start=(k == 0), stop=(k == 1),
                                )
                        nc.vector._custom_dve(
                            qpack,
                            out=trash[:],
                            in0=sps[:],
                            in1=se2b[:, db, g * 2048:(g + 1) * 2048],
                            imm2=BIG,
                            accum_out=pk64[:, rt * 4 + g: rt * 4 + g + 1],
                        )
                # index extraction for this quarter
                cs = qq * 16
                pk3 = pk64[:, cs:cs + 16].rearrange("p (a b) -> p a b", a=4)
                m16 = small_p.tile([128, 4], f32, name="m16", tag="m16")
                nc.vector.tensor_reduce(
                    m16[:], pk3, axis=mybir.AxisListType.X, op=AluOpType.max
                )
                msk = small_p.tile([128, 4, 4], f32, name="msk", tag="msk")
                nc.vector.tensor_tensor(
                    msk[:], pk3,
                    m16[:].rearrange("p (a o) -> p a o", o=1)
                    .broadcast_to((128, 4, 4)),
                    op=AluOpType.is_ge,
                )
                nc.vector.tensor_mul(
                    msk[:], msk[:],
                    giota[:, cs:cs + 16].rearrange("p (a b) -> p a b", a=4),
                )
                gidx = small_p.tile([128, 4], f32, name="gidx", tag="gidx")
                nc.vector.tensor_reduce(
                    gidx[:], msk[:], axis=mybir.AxisListType.X, op=AluOpType.add
                )
                nc.vector.tensor_scalar_min(gidx[:], gidx[:], 3.0)
                mi = small_p.tile([128, 4], mybir.dt.int32, name="mi", tag="mi")
                nc.vector.tensor_copy(mi[:], m16[:])
                nc.vector.tensor_scalar(
                    mi[:], mi[:], int(GRID) - 1, None, op0=AluOpType.bitwise_and
                )
                loc = small_p.tile([128, 4], f32, name="loc", tag="loc")
                nc.vector.tensor_copy(loc[:], mi[:])
                nc.vector.tensor_scalar(
                    gidx[:], gidx[:], GRID, None, op0=AluOpType.mult
                )
                nc.vector.tensor_add(loc[:], loc[:], gidx[:])
                nc.vector.tensor_copy(idx16[:, qq * 4:(qq + 1) * 4], loc[:])
                # stage indices (wrapped + replicated across 8 Q7 groups)
                for kk in range(8):
                    nc.gpsimd.dma_start(
                        idxg[0:16, qq * 4:(qq + 1) * 4, kk],
                        idx16[kk * 16:(kk + 1) * 16, qq * 4:(qq + 1) * 4],
                    )
                for gg in range(1, 8):
                    nc.gpsimd.dma_start(
                        idxg[gg * 16:(gg + 1) * 16, qq * 4:(qq + 1) * 4, :],
                        idxg[0:16, qq * 4:(qq + 1) * 4, :],
                    )
                nc.gpsimd.ap_gather(
                    qp[:, qq * 512:(qq + 1) * 512],
                    e2tp[:],
                    idxg[:, qq * 4:(qq + 1) * 4, :].rearrange("p a b -> p (a b)"),
                    channels=128, num_elems=VOCAB, d=1, num_idxs=512,
                )
                # apply the PREVIOUS quarter's update here so the in-order
                # DVE never waits on this quarter's gather
                for uq in ([qq - 1] if qq > 0 else []) + ([qq] if qq == 3 else []):
                    for m in range(2):
                        nc.vector.tensor_sub(
                            residT[:, m, uq * 512:(uq + 1) * 512]
                            .rearrange("p (n o) -> p n o", o=1),
                            residT[:, m, uq * 512:(uq + 1) * 512]
                            .rearrange("p (n o) -> p n o", o=1),
                            qb[:, uq * 512:(uq + 1) * 512, m:m + 1],
                        )
                        if lv < HQ - 1:
                            nc.vector.tensor_copy(
                                residTb[:, m, uq * 512:(uq + 1) * 512],
                                residT[:, m, uq * 512:(uq + 1) * 512],
                            )
            for m in range(2):
                nc.scalar.activation(
                    trash[:], residT[:, m, :], Square,
                    accum_out=out_sb[:, 2 * lv + m: 2 * lv + m + 1],
                )

        for m in range(2):
            # quant^T = latT - residT (stored back into latT)
            nc.vector.tensor_sub(latT[:, m, :], latT[:, m, :], residT[:, m, :])

        vq_ctx.close()

        # =============== decoder ===============
        dec_ctx = contextlib.ExitStack()
        dec_p = dec_ctx.enter_context(tc.tile_pool(name="decp", bufs=1))
        work_p = dec_ctx.enter_context(tc.tile_pool(name="decw", bufs=2))
        dps_p = dec_ctx.enter_context(tc.tile_pool(name="dps", bufs=4, space="PSUM"))
        dw1s = dec_p.tile([128, 2, HID], bf16, name="dw1s")
        quantTb = dec_p.tile([128, 2, NSH], bf16, name="quantTb")
        for m in range(2):
            nc.vector.tensor_copy(quantTb[:, m, :], latT[:, m, :])
        for k in range(2):
            nc.sync.dma_start(dw1s[:, k, :], dw1_d.ap()[k * 128:(k + 1) * 128, :])
        dw2s = dec_p.tile([128, HID // 128, OBS], bf16, name="dw2s")
        for k in range(HID // 128):
            nc.sync.dma_start(dw2s[:, k, :], dw2_d.ap()[k * 128:(k + 1) * 128, :])
        db1s = const_p.tile([HID // 512, 512], bf16, name="db1s")
        if use_db1:
            nc.sync.dma_start(db1s[:], db1_d.ap())
        db2s = const_p.tile([OBS // 512, 512], bf16, name="db2s")
        if use_db2:
            nc.sync.dma_start(db2s[:], db2_d.ap())

        for rc in range(4):  # row chunks of 512
            dhT = work_p.tile([128, HID // 128, 512], bf16, name="dhT", tag="hT")
            for ht in range(HID // 128):
                dps = dps_p.tile([128, 512], f32, name="dps", tag="dmm")
                for k in range(2):
                    nc.tensor.matmul(
                        dps[:, 0:512],
                        dw1s[:, k, ht * 128:(ht + 1) * 128],
                        quantTb[:, k, rc * 512:(rc + 1) * 512],
                        start=(k == 0), stop=(k == 1 and not use_db1),
                    )
                if use_db1:
                    nc.tensor.matmul(
                        dps[:, 0:512],
                        db1s[(ht * 128) // 512:(ht * 128) // 512 + 1,
                             (ht * 128) % 512:(ht * 128) % 512 + 128],
                        ones1[:],
                        start=False, stop=True,
                    )
                nc.scalar.activation(dhT[:, ht, :], dps[:, 0:512], Relu)
            for ot in range(OBS // 128):
                xTl = work_p.tile([128, 512], bf16, name="xTl", tag="xTl")
                nc.sync.dma_start(
                    xTl[:],
                    x_d.ap()[ot * 128:(ot + 1) * 128, rc * 512:(rc + 1) * 512],
                )
                rps = dps_p.tile([128, 512], f32, name="rps", tag="dmm")
                nk = HID // 128
                for k in range(nk):
                    nc.tensor.matmul(
                        rps[:, 0:512],
                        dw2s[:, k, ot * 128:(ot + 1) * 128],
                        dhT[:, k, :],
                        start=(k == 0), stop=(k == nk - 1 and not use_db2),
                    )
                if use_db2:
                    nc.tensor.matmul(
                        rps[:, 0:512],
                        db2s[(ot * 128) // 512:(ot * 128) // 512 + 1,
                             (ot * 128) % 512:(ot * 128) % 512 + 128],
                        ones1[:],
                        start=False, stop=True,
                    )
                diff = work_p.tile([128, 512], f32, name="diff", tag="diff")
                nc.vector.tensor_sub(diff[:], rps[:, 0:512], xTl[:])
                nc.scalar.activation(
                    diff[:], diff[:], Square,
                    accum_out=out_sb[:, 8 + rc * 8 + ot: 9 + rc * 8 + ot],
                )

        dec_ctx.close()
        nc.sync.dma_start(out_d.ap(), out_sb[:])

    nc.compile()
    return nc


def _host_prep(inputs):
    import ml_dtypes

    x = np.asarray(inputs["x"], np.float32)
    cb = np.ascontiguousarray(np.asarray(inputs["codebooks"], np.float32))
    w1 = np.ascontiguousarray(np.asarray(inputs["enc_w1"], np.float32))
    b1 = np.asarray(inputs["enc_b1"], np.float32)
    lng = np.asarray(inputs["ln_g"], np.float32)
    lnb = np.asarray(inputs["ln_b"], np.float32)
    w2 = np.asarray(inputs["enc_w2"], np.float32)
    b2 = np.asarray(inputs["enc_b2"], np.float32)
    dw1 = np.ascontiguousarray(np.asarray(inputs["dec_w1"], np.float32))
    db1 = np.asarray(inputs["dec_b1"], np.float32)
    dw2 = np.asarray(inputs["dec_w2"], np.float32)
    db2 = np.asarray(inputs["dec_b2"], np.float32)

    assert np.all(lnb == 0.0) and np.all(lng > 0.0), "kernel assumes ln_b==0, ln_g>0"
    w2g = w2 * lng[:, None]  # relu(z*g)@W2 == relu(z)@(g[:,None]*W2) for g>0

    # sample-estimate per-level score ranges to pick K, SHIFT
    rng = np.random.default_rng(0)
    sel = rng.choice(x.shape[0], 256, replace=False)
    h = x[sel] @ w1 + b1
    mu = h.mean(-1, keepdims=True)
    var = ((h - mu) ** 2).mean(-1, keepdims=True)
    h = np.maximum((h - mu) / np.sqrt(var + LN_EPS) * lng + lnb, 0.0)
    resid = h @ w2 + b2
    e2sum = (cb.astype(np.float64) ** 2).sum(-1).astype(np.float32)  # [HQ, VOCAB]
    Ks, SHIFTs = [], []
    for lv in range(HQ):
        sc = 2.0 * resid @ cb[lv].T - e2sum[lv]
        lo, hi = float(sc.min()), float(sc.max())
        span = hi - lo
        shift = -lo + 0.75 * span + 16.0       # margin: scores stay well positive
        smax = (hi + shift) * 2.0              # 2x safety for sample underestimate
        K = np.float32((2.0**24 * 0.98) / smax)
        Ks.append(K)
        SHIFTs.append(np.float32(shift))
        idx = sc.argmax(-1)
        resid = resid - cb[lv][idx]

    e2t = cb.transpose(0, 2, 1)  # [HQ, LAT, VOCAB]
    e2t_bf = np.zeros((HQ, LAT, VOCAB), ml_dtypes.bfloat16)
    se2 = np.zeros((HQ, 128, VOCAB), ml_dtypes.bfloat16)
    e2tp_pack = np.zeros((HQ, 128, VOCAB), np.uint32)
    for lv in range(HQ):
        e2t_bf[lv] = (np.float32(2.0 * Ks[lv]) * e2t[lv]).astype(ml_dtypes.bfloat16)
        row = (Ks[lv] * (SHIFTs[lv] - e2sum[lv])).astype(ml_dtypes.bfloat16)
        se2[lv] = np.broadcast_to(row, (128, VOCAB))
        pk0 = e2t[lv, :128].astype(ml_dtypes.bfloat16).view(np.uint16).astype(np.uint32)
        pk1 = e2t[lv, 128:].astype(ml_dtypes.bfloat16).view(np.uint16).astype(np.uint32)
        e2tp_pack[lv] = pk0 | (pk1 << 16)

    common = {
        "w1b": np.ascontiguousarray(w1.astype(ml_dtypes.bfloat16)),
        "b1": np.ascontiguousarray(b1.reshape(HID // 512, 512).astype(ml_dtypes.bfloat16)),
        "w2b": np.ascontiguousarray(w2g.astype(ml_dtypes.bfloat16)),
        "b2": b2.reshape(1, LAT).astype(ml_dtypes.bfloat16),
        "dw1b": np.ascontiguousarray(dw1.astype(ml_dtypes.bfloat16)),
        "db1": np.ascontiguousarray(db1.reshape(HID // 512, 512).astype(ml_dtypes.bfloat16)),
        "dw2b": np.ascontiguousarray(dw2.astype(ml_dtypes.bfloat16)),
        "db2": np.ascontiguousarray(db2.reshape(OBS // 512, 512).astype(ml_dtypes.bfloat16)),
        "e2t": np.ascontiguousarray(e2t_bf),
        "se2": np.ascontiguousarray(se2),
        "e2tp": e2tp_pack,
        "identb": np.eye(128, dtype=np.float32).astype(ml_dtypes.bfloat16),
        "giota": np.ascontiguousarray(
            np.tile(np.arange(4, dtype=np.float32), (128, RT))
        ),
    }
    flags = dict(
        use_b1=bool(np.any(b1 != 0)),
        use_b2=bool(np.any(b2 != 0)),
        use_db1=bool(np.any(db1 != 0)),
        use_db2=bool(np.any(db2 != 0)),
    )
    in_maps = []
    for c in range(NCORES):
        m = dict(common)
        m["xbt"] = np.ascontiguousarray(
            x[c * NSH:(c + 1) * NSH].T.astype(ml_dtypes.bfloat16)
        )
        in_maps.append(m)
    return in_maps, flags


def _combine(results):
    rlv = rrec = 0.0
    for c in range(NCORES):
        o = np.asarray(results[c]["out"], np.float64)
        rlv += o[:, 0:8].sum()
        rrec += o[:, 8:40].sum()
    return np.float32(1.5 * rlv / (N * LAT) + 0.5 * rrec / (N * OBS))


_NC_CACHE = {}


def get_nc(flags):
    key = tuple(sorted(flags.items()))
    if key not in _NC_CACHE:
        _NC_CACHE[key] = build_nc(**flags)
    return _NC_CACHE[key]


def kernel(**inputs) -> np.ndarray:
    in_maps, flags = _host_prep(inputs)
    nc = get_nc(flags)
    res = run_bass_kernel_spmd(nc, in_maps, core_ids=list(range(NCORES)))
    return _combine(res.results)
